# revision 4
# baseline (speedup 1.0000x reference)
"""Trainium2 Bass kernel for nn_BlockAttention (topk channel gather).

reference:
    y = mean(x, spatial)                      # [b, 256]
    g = sigmoid(relu(y @ W1.T) @ W2.T)        # [b, 256]
    idx = top_k(g, 16)                        # [b, 16]
    out = x[b, idx]                           # [b, 16, 128, 128]

Ranking is invariant under positive scaling (relu positively homogeneous,
matmul linear) and under sigmoid (strictly monotonic), so the kernel ranks
raw channel sums pushed through relu-MLP without the /HW and sigmoid.

Sharding: data-parallel over batch, 4 samples per core on 8 cores; W1/W2
replicated. No cross-core communication.
"""

import numpy as np

import concourse.bacc as bacc
import concourse.bass as bass
import concourse.mybir as mybir
import concourse.tile as tile
from concourse.bass_utils import run_bass_kernel_spmd
from concourse.masks import make_identity

F32 = mybir.dt.float32
U32 = mybir.dt.uint32
P = 128

N_CORES = 8
B = 32          # global batch
B_LOC = B // N_CORES
C = 256         # channels
H = W_SP = 128  # spatial
HW = H * W_SP   # 16384
TOPK = 16

NCH = C // P    # channel halves: 2
SCH = 4096      # spatial chunk per streamed tile
NSC = HW // SCH # 4
RPC = 8         # gather rows per channel (channel slab split into 8x2048)
RL = HW // RPC  # 2048 row length


def build_kernel() -> bacc.Bacc:
    nc = bacc.Bacc(name="blockattn_topk")

    x = nc.declare_dram_parameter("x", [B_LOC, C, H, W_SP], F32, isOutput=False)
    w1 = nc.declare_dram_parameter("w1", [C, C], F32, isOutput=False)
    w2 = nc.declare_dram_parameter("w2", [C, C], F32, isOutput=False)
    out = nc.declare_dram_parameter("out", [B_LOC, TOPK, H, W_SP], F32, isOutput=True)

    x_v = x[:].rearrange("b c h w -> b c (h w)")                 # [4, 256, 16384]
    x_tab = x[:].rearrange("b c (r q) w -> (b c r) (q w)", r=RPC)  # [8192, 2048]
    out_v = out[:].rearrange("b k (r q) w -> b (k r) (q w)", r=RPC)  # [4, 128, 2048]

    with tile.TileContext(nc) as tc:
        with (
            tc.tile_pool(name="persist", bufs=1) as pers,
            tc.tile_pool(name="stream", bufs=8) as sp,
            tc.tile_pool(name="psum", bufs=1, space="PSUM") as pp,
            tc.tile_pool(name="gather", bufs=2) as gp,
        ):
            # ---- weights: load and PE-transpose to [c_in, c_out] layout ----
            ident = pers.tile([P, P], F32, tag="ident")
            make_identity(nc, ident[:])

            w1n = [pers.tile([P, C], F32, tag=f"w1n{r}", name=f"w1n{r}") for r in range(NCH)]
            w2n = [pers.tile([P, C], F32, tag=f"w2n{r}", name=f"w2n{r}") for r in range(NCH)]
            w1t = [pers.tile([P, C], F32, tag=f"w1t{c}", name=f"w1t{c}") for c in range(NCH)]
            w2t = [pers.tile([P, C], F32, tag=f"w2t{c}", name=f"w2t{c}") for c in range(NCH)]
            for r in range(NCH):
                nc.sync.dma_start(out=w1n[r][:], in_=w1[r * P:(r + 1) * P, :])
                nc.sync.dma_start(out=w2n[r][:], in_=w2[r * P:(r + 1) * P, :])
            for wn, wt in ((w1n, w1t), (w2n, w2t)):
                for r in range(NCH):
                    for cb in range(NCH):
                        pt = pp.tile([P, P], F32, tag="tpose")
                        nc.tensor.transpose(
                            out=pt[:], in_=wn[r][:, cb * P:(cb + 1) * P],
                            identity=ident[:],
                        )
                        nc.vector.tensor_copy(
                            out=wt[cb][:, r * P:(r + 1) * P], in_=pt[:]
                        )

            # ---- phase A: stream x, per-channel spatial sums ----
            acc = [pers.tile([P, B_LOC * NSC], F32, tag=f"acc{c}", name=f"acc{c}") for c in range(NCH)]
            y = [pers.tile([P, B_LOC], F32, tag=f"y{c}", name=f"y{c}") for c in range(NCH)]
            for b in range(B_LOC):
                for ch in range(NCH):
                    for s in range(NSC):
                        t = sp.tile([P, SCH], F32, tag="stream")
                        nc.sync.dma_start(
                            out=t[:],
                            in_=x_v[b, ch * P:(ch + 1) * P, s * SCH:(s + 1) * SCH],
                        )
                        nc.vector.reduce_sum(
                            out=acc[ch][:, b * NSC + s:b * NSC + s + 1],
                            in_=t[:], axis=mybir.AxisListType.X,
                        )
            for ch in range(NCH):
                nc.vector.reduce_sum(
                    out=y[ch][:],
                    in_=acc[ch][:].rearrange("p (b s) -> p b s", s=NSC),
                    axis=mybir.AxisListType.X,
                )

            # ---- phase B: gate MLP (no bias), relu between, no sigmoid ----
            hrel = [pers.tile([P, B_LOC], F32, tag=f"h{j}", name=f"h{j}") for j in range(NCH)]
            sc = [pers.tile([P, B_LOC], F32, tag=f"sc{o}", name=f"sc{o}") for o in range(NCH)]
            for jh in range(NCH):
                z1 = pp.tile([P, B_LOC], F32, tag="z1")
                for ch in range(NCH):
                    nc.tensor.matmul(
                        out=z1[:], lhsT=w1t[ch][:, jh * P:(jh + 1) * P],
                        rhs=y[ch][:], start=(ch == 0), stop=(ch == NCH - 1),
                    )
                nc.scalar.activation(
                    out=hrel[jh][:], in_=z1[:], func=mybir.ActivationFunctionType.Relu
                )
            for oh in range(NCH):
                z2 = pp.tile([P, B_LOC], F32, tag="z2")
                for jh in range(NCH):
                    nc.tensor.matmul(
                        out=z2[:], lhsT=w2t[jh][:, oh * P:(oh + 1) * P],
                        rhs=hrel[jh][:], start=(jh == 0), stop=(jh == NCH - 1),
                    )
                nc.vector.tensor_copy(out=sc[oh][:], in_=z2[:])

            # ---- phase C: transpose scores to [b, 256]; top-16 via 2x max8 ----
            stp = pp.tile([B_LOC, C], F32, tag="stp")
            for oh in range(NCH):
                nc.tensor.transpose(
                    out=stp[:, oh * P:(oh + 1) * P], in_=sc[oh][:], identity=ident[:]
                )
            st = pers.tile([B_LOC, C], F32, tag="st")
            nc.vector.tensor_copy(out=st[:], in_=stp[:])

            m1v = pers.tile([B_LOC, 8], F32, tag="m1v")
            m2v = pers.tile([B_LOC, 8], F32, tag="m2v")
            msk = pers.tile([B_LOC, C], F32, tag="msk")
            idxu = pers.tile([B_LOC, TOPK], U32, tag="idxu")
            nc.vector.max(out=m1v[:], in_=st[:])
            nc.vector.max_index(out=idxu[:, 0:8], in_max=m1v[:], in_values=st[:])
            nc.vector.match_replace(
                out=msk[:], in_to_replace=m1v[:], in_values=st[:], imm_value=-1e30
            )
            nc.vector.max(out=m2v[:], in_=msk[:])
            nc.vector.max_index(out=idxu[:, 8:16], in_max=m2v[:], in_values=msk[:])

            # ---- phase D: expand to row indices, transpose to partitions, gather ----
            # row(b, k, j) = b*2048 + idx[b,k]*8 + j  into x_tab [8192, 2048]
            idxe = pers.tile([B_LOC, P], U32, tag="idxe")
            idxe_v = idxe[:].rearrange("b (k j) -> b k j", j=RPC)
            for j in range(RPC):
                nc.vector.tensor_scalar(
                    out=idxe_v[:, :, j], in0=idxu[:], scalar1=RPC, scalar2=j,
                    op0=mybir.AluOpType.mult, op1=mybir.AluOpType.add,
                )
            boff = pers.tile([B_LOC, 1], U32, tag="boff")
            nc.gpsimd.iota(
                out=boff[:], pattern=[[0, 1]], base=0,
                channel_multiplier=C * RPC,
            )
            nc.vector.tensor_tensor(
                out=idxe[:], in0=idxe[:], in1=boff[:].to_broadcast([B_LOC, P]),
                op=mybir.AluOpType.add,
            )
            idxf = pers.tile([B_LOC, P], F32, tag="idxf")
            nc.vector.tensor_copy(out=idxf[:], in_=idxe[:])
            ridxp = pp.tile([P, B_LOC], F32, tag="ridxp")
            nc.tensor.transpose(
                out=ridxp[:], in_=idxf[:], identity=ident[0:B_LOC, 0:B_LOC]
            )
            ridx = pers.tile([P, B_LOC], mybir.dt.int32, tag="ridx")
            nc.vector.tensor_copy(out=ridx[:], in_=ridxp[:])

            for b in range(B_LOC):
                g = gp.tile([P, RL], F32, tag="g")
                nc.gpsimd.indirect_dma_start(
                    out=g[:], out_offset=None, in_=x_tab,
                    in_offset=bass.IndirectOffsetOnAxis(ap=ridx[:, b:b + 1], axis=0),
                )
                nc.sync.dma_start(out=out_v[b], in_=g[:])

    nc.compile()
    return nc


def kernel(x: np.ndarray, W1: np.ndarray, W2: np.ndarray) -> np.ndarray:
    assert x.shape == (B, C, H, W_SP), x.shape
    nc = build_kernel()
    in_maps = [
        {
            "x": np.ascontiguousarray(x[i * B_LOC:(i + 1) * B_LOC]),
            "w1": np.ascontiguousarray(W1),
            "w2": np.ascontiguousarray(W2),
        }
        for i in range(N_CORES)
    ]
    res = run_bass_kernel_spmd(nc, in_maps, core_ids=list(range(N_CORES)))
    return np.concatenate([res.results[i]["out"] for i in range(N_CORES)], axis=0)
